# revision 15
# baseline (speedup 1.0000x reference)
"""Alpha-filter (keras_spiking AlphaCell) Trainium2 Bass kernel — matmul scan.

Math: per (batch b, feature k) the reference runs the 2-state recurrence
    x_t = A_k x_{t-1} + B_k u_t,   y_t = x_t[1]
which reduces (see kernel_baseline.py) to the causal convolution
    y_t = sum_{j<=t} h_{t-j} u_j + carry,   h_m = cS e^m + cEta m e^{m-1}.

Each 128-step time block is computed with PE matmuls via the separable
factorization (input rows i, output rows r, features k on the free dim):
    h_{r-i} = e^r e^{-i} [ cS + cEta' (r-i) ],   cEta' = cEta/e
    v1 = (cS e^{-i}) (.) u,  v2 = (cEta' e^{-i}) (.) u     (prescales)
    P  = Tril^T v1 + TrilM^T v2 + Wr^T R    (fp32r matmuls, one PSUM bank)
    y  = E (.) P                            (elementwise combine)
with Tril[i,r] = [i<=r], TrilM[i,r] = (r-i)[i<=r] (the (r-i) factor lives
entirely in the weights — no large-coefficient cancellation), E[r,k] =
e^r, and R the 2-row carry inject (rows r1 = e cEta ce + (e cS+cEta) cs,
r2 = cEta cs; y += E (r1 + r r2)).

Cross-block carries (cs = s-state, ce = eta-state entering each block):
per-block summary matmuls accumulate Sm0 = sum_i v2, Sm1 = sum_i i v2
into a [16,512] PSUM tile per batch; batches are processed in groups of
2, and per group ONE transposed layout [128k, (8 bc)(2 s)(8 m)]
(bc = 4*b_loc + c) feeds a fully batched carry chain:
    cs' = e^L cs + k1 Sm0,   ce' = e^L ce + bL cs + k2 Sm0 - k3 Sm1
done with ~12 wide DVE ops + 2 reset-trick scans (data0 = 0 at every
m = 0 kills the carry across (b, c) boundaries, and scan inputs are
shifted one slot so the scans directly emit the carry INTO each block).
The resulting R rows are transposed back per batch for the injects.

Engines (measured rates): DVE does f32 prescales (0.65 ns/row), the
chain (~2 us per group), and 4-block-merged in-place combines from SBUF;
Act drains PSUM->SBUF (single-bank copies — multi-bank PSUM reads on DVE
are ~6x slow); PE does fp32r matmuls; GpSimd issues out-DMAs.  No data
transposes anywhere — inputs stay in natural [time, feature] layout.

Sharding: data-parallel over batch, 8 batches per core x 8 cores.
"""

import sys

for _p in ("/opt/trn_rl_repo",):
    if _p not in sys.path:
        sys.path.insert(0, _p)

from contextlib import ExitStack

import numpy as np

import concourse.bacc as bacc
import concourse.bass as bass
import concourse.tile as tile
from concourse import mybir
from concourse.bass_utils import run_bass_kernel_spmd

DT = 0.001
B, T, K = 64, 1024, 512
N_CORES = 8
B_LOC = B // N_CORES  # 8 batches per core
P = 128
NBLK = T // P   # 8 time blocks of 128
KC = K // P     # 4 feature chunks of 128 (carry chain only)
HB = 2          # batches per chain group
NG = B_LOC // HB  # 4 chain groups
NBC = HB * KC   # 8 (b_loc, c) pairs per group

F32 = mybir.dt.float32
F32R = mybir.dt.float32r
MULT = mybir.AluOpType.mult
ADD = mybir.AluOpType.add
SUB = mybir.AluOpType.subtract


def _ap(base, off_elems, dims):
    """Custom AP: base tile AP -> new free dims (keeps partition dim)."""
    return bass.AP(tensor=base.tensor, offset=base.offset + off_elems,
                   ap=[base.ap[0]] + dims)


def build_nc():
    nc = bacc.Bacc(None, target_bir_lowering=False)

    x = nc.dram_tensor("x", [B_LOC, T, K], F32, kind="ExternalInput")
    e1 = nc.dram_tensor("e1", [P, K], F32, kind="ExternalInput")
    gc = nc.dram_tensor("gc", [P, 2 * K], F32, kind="ExternalInput")
    wmat = nc.dram_tensor("wmat", [P, 3 * P], F32R, kind="ExternalInput")
    wr = nc.dram_tensor("wr", [16, NBLK * P], F32R, kind="ExternalInput")
    ident = nc.dram_tensor("ident", [P, P], F32, kind="ExternalInput")
    chc = nc.dram_tensor("chc", [P, 11 * NBC + NBC * NBLK], F32,
                         kind="ExternalInput")
    y = nc.dram_tensor("y", [B_LOC, T, K], F32, kind="ExternalOutput")

    with tile.TileContext(nc) as tc, ExitStack() as ctx:
        singles = ctx.enter_context(tc.tile_pool(name="singles", bufs=1))
        inpool = ctx.enter_context(tc.tile_pool(name="inpool", bufs=3))
        vpool = ctx.enter_context(tc.tile_pool(name="vpool", bufs=3))
        smpool = ctx.enter_context(tc.tile_pool(name="smpool", bufs=2))
        chpool = ctx.enter_context(tc.tile_pool(name="chpool", bufs=2))
        rpool = ctx.enter_context(tc.tile_pool(name="rpool", bufs=4))
        stagepool = ctx.enter_context(tc.tile_pool(name="stagepool", bufs=3))
        ppool = ctx.enter_context(tc.tile_pool(name="ppool", bufs=2, space="PSUM"))
        smps = ctx.enter_context(tc.tile_pool(name="smps", bufs=1, space="PSUM"))
        stps = ctx.enter_context(tc.tile_pool(name="stps", bufs=2, space="PSUM"))
        rps = ctx.enter_context(tc.tile_pool(name="rps", bufs=1, space="PSUM"))

        # ---- one-time constant loads -----------------------------------
        e1_t = singles.tile([P, K], F32)
        nc.scalar.dma_start(out=e1_t[:], in_=e1[:])
        gc_t = singles.tile([P, 2 * K], F32)
        nc.scalar.dma_start(out=gc_t[:], in_=gc[:])
        wmat_t = singles.tile([P, 3 * P], F32R)
        nc.scalar.dma_start(out=wmat_t[:], in_=wmat[:])
        wr_t = singles.tile([16, NBLK * P], F32R)
        nc.scalar.dma_start(out=wr_t[:], in_=wr[:])
        ident_t = singles.tile([P, P], F32)
        nc.scalar.dma_start(out=ident_t[:], in_=ident[:])
        chc_t = singles.tile([P, 11 * NBC + NBC * NBLK], F32)
        nc.scalar.dma_start(out=chc_t[:], in_=chc[:])

        # PE warm-up during the initial DMA window (HAM clock ramp).
        scratch = singles.tile([P, P], F32)
        nc.gpsimd.memset(scratch[:], 0.0)
        scrb = singles.tile([P, 16], mybir.dt.bfloat16)
        nc.gpsimd.memset(scrb[:], 0.0)

        def pe_tickle():
            # keep the PE activity monitor fed so the clock stays boosted
            nc.tensor.ldweights(scrb[:])
        warm = ppool.tile([P, 2, K], F32, name="warm", tag="pt")
        for _ in range(48):
            nc.tensor.matmul(warm[:, 0, 0:P], scratch[:], scratch[:],
                             start=True, stop=True)

        chb = chc_t[:]

        def cc(j):
            return chc_t[:, j * NBC:(j + 1) * NBC]

        def cbc(j, n=NBLK):
            return _ap(chb, j * NBC, [[1, NBC], [0, n]])

        D0OFF = 11 * NBC
        d0_2d = chc_t[:, D0OFF:D0OFF + NBC * NBLK]

        w_tril = wmat_t[:, 0:P]
        w_trilM = wmat_t[:, P: 2 * P]
        e1b = _ap(e1_t[:], 0, [[0, 4], [1, K]])

        vs = {}
        rsb = {}
        sts = {}

        def emit_A1(b):
            """DMA-in + prescales (DVE)."""
            xv = x[b].rearrange("(a p) k -> p a k", p=P)
            v12 = vpool.tile([P, 2, NBLK, K], F32R, tag="v12", name=f"v12_{b}")
            v12b = v12[:]
            for h in (0, 4):
                in_h = inpool.tile([P, 4, K], F32, tag="in",
                                   name=f"in{b}_{h}")
                nc.sync.dma_start(out=in_h[:], in_=xv[:, h:h + 4, :])
                for w in (0, 1):
                    nc.vector.tensor_tensor(
                        out=_ap(v12b, w * NBLK * K + h * K, [[1, 4 * K]]),
                        in0=in_h[:],
                        in1=_ap(gc_t[:], w * K, [[0, 4], [1, K]]),
                        op=MULT)
            vs[b] = v12

        def emit_A2(b):
            """Summary matmuls, SmAll copy, ST transpose (PE + Act)."""
            if b % HB == 0:
                sts[b // HB] = stps.tile([P, NBC * 16], F32, tag="st",
                                         name=f"st{b // HB}")
            st_ps = sts[b // HB]
            b_loc = b % HB
            v12 = vs[b]
            sm_ps = smps.tile([16, K], F32)
            for m in range(NBLK):
                nc.tensor.matmul(
                    sm_ps[:],
                    wmat_t[:, 2 * P + 16 * m: 2 * P + 16 * (m + 1)],
                    v12[:, 1, m, :],
                    start=(m == 0), stop=(m == NBLK - 1))
            pe_tickle()
            sm_sb = smpool.tile([16, K], F32)
            nc.scalar.copy(sm_sb[:], sm_ps[:])
            for c in range(KC):
                bc = b_loc * KC + c
                nc.tensor.matmul(st_ps[:, bc * 16:(bc + 1) * 16],
                                 sm_sb[:, c * P:(c + 1) * P],
                                 ident_t[0:16, 0:16], is_transpose=True,
                                 skip_group_check=True)

        def emit_H(grp):
            """Batched carry chain for HB batches -> R rows per batch."""
            st_ps = sts[grp]
            stb = st_ps[:]

            in1 = chpool.tile([P, NBC * NBLK], F32, tag="in1")
            in2 = chpool.tile([P, NBC * NBLK], F32, tag="in2")
            t1 = chpool.tile([P, NBC * NBLK], F32, tag="t1")
            d2 = chpool.tile([P, NBC * NBLK], F32, tag="d2")
            csp = chpool.tile([P, NBC * NBLK], F32, tag="csp")
            cep = chpool.tile([P, NBC * NBLK], F32, tag="cep")
            rt = chpool.tile([P, NBC, 2, NBLK], F32, tag="rt")
            in1b, in2b, t1b, d2b = in1[:], in2[:], t1[:], d2[:]
            cs_prev = _ap(csp[:], 0, [[NBLK, NBC], [1, NBLK]])
            ce_prev = _ap(cep[:], 0, [[NBLK, NBC], [1, NBLK]])
            # shifted views: scan position m holds inputs of step m-1;
            # position 0 holds the initial carry (reset trick: d0 = 0 there)
            sm0s = _ap(stb, 0, [[16, NBC], [1, NBLK - 1]])
            sm1s = _ap(stb, 8, [[16, NBC], [1, NBLK - 1]])

            def shifted(b_):
                return _ap(b_, 1, [[NBLK, NBC], [1, NBLK - 1]])

            def col0(b_):
                return _ap(b_, 0, [[NBLK, NBC]])

            nc.vector.tensor_tensor(out=shifted(in1b), in0=sm0s,
                                    in1=cbc(0, NBLK - 1), op=MULT)
            nc.scalar.copy(col0(in1b), cc(9))
            nc.vector.tensor_tensor(out=shifted(in2b), in0=sm0s,
                                    in1=cbc(1, NBLK - 1), op=MULT)
            nc.vector.tensor_tensor(out=shifted(t1b), in0=sm1s,
                                    in1=cbc(2, NBLK - 1), op=MULT)
            nc.vector.tensor_tensor(out=shifted(in2b), in0=shifted(in2b),
                                    in1=shifted(t1b), op=SUB)
            nc.vector.tensor_tensor_scan(
                out=csp[:], data0=d0_2d, data1=in1[:],
                initial=0.0, op0=MULT, op1=ADD)
            nc.vector.tensor_tensor(
                out=shifted(d2b),
                in0=_ap(csp[:], 0, [[NBLK, NBC], [1, NBLK - 1]]),
                in1=cbc(3, NBLK - 1), op=MULT)
            nc.vector.tensor_tensor(out=shifted(d2b), in0=shifted(d2b),
                                    in1=shifted(in2b), op=ADD)
            nc.scalar.copy(col0(d2b), cc(10))
            nc.vector.tensor_tensor_scan(
                out=cep[:], data0=d0_2d, data1=d2[:],
                initial=0.0, op0=MULT, op1=ADD)
            # r1 = mu1*ce_prev + mu2*cs_prev ; r2 = nu*cs_prev
            nc.vector.tensor_tensor(out=t1[:], in0=ce_prev, in1=cbc(4), op=MULT)
            nc.vector.tensor_tensor(out=in1[:], in0=cs_prev, in1=cbc(5), op=MULT)
            nc.vector.tensor_tensor(
                out=_ap(rt[:], 0, [[2 * NBLK, NBC], [1, NBLK]]),
                in0=t1[:], in1=in1[:], op=ADD)
            nc.vector.tensor_tensor(
                out=_ap(rt[:], NBLK, [[2 * NBLK, NBC], [1, NBLK]]),
                in0=cs_prev, in1=cbc(6), op=MULT)

            for b_loc in range(HB):
                b = grp * HB + b_loc
                r_ps = rps.tile([16, K], F32, tag="rps", name=f"rps{b}")
                for c in range(KC):
                    bc = b_loc * KC + c
                    nc.tensor.matmul(r_ps[:, c * P:(c + 1) * P],
                                     rt[:, bc, :, :], ident_t[:],
                                     is_transpose=True, skip_group_check=True)
                r_sb = rpool.tile([16, K], F32R, tag="rsb", name=f"rsb{b}")
                nc.scalar.copy(r_sb[:], r_ps[:])
                rsb[b] = r_sb

        def emit_C(b):
            """Block matmuls, Act PSUM drain, in-place combine, DMA out."""
            v12 = vs.pop(b)
            r_sb = rsb.pop(b)
            yv = y[b].rearrange("(a p) k -> p a k", p=P)
            for g in (0, 1):
                stage = stagepool.tile([P, 4, K], F32, tag="stage",
                                       name=f"sg{b}_{g}")
                for jj in (0, 1):
                    pt = ppool.tile([P, 2, K], F32, tag="pt",
                                    name=f"pt{b}_{g}_{jj}")
                    for half_i in (0, 1):
                        m = 4 * g + 2 * jj + half_i
                        nc.tensor.matmul(pt[:, half_i, :], w_tril,
                                         v12[:, 0, m, :],
                                         start=True, stop=False,
                                         skip_group_check=True)
                        nc.tensor.matmul(pt[:, half_i, :], w_trilM,
                                         v12[:, 1, m, :],
                                         start=False, stop=False,
                                         skip_group_check=True)
                        nc.tensor.matmul(pt[:, half_i, :],
                                         wr_t[:, m * P:(m + 1) * P],
                                         r_sb[:],
                                         start=False, stop=True,
                                         skip_group_check=True)
                        nc.scalar.copy(stage[:, 2 * jj + half_i, :],
                                       pt[:, half_i, :])
                    pe_tickle()
                nc.vector.tensor_tensor(out=stage[:], in0=stage[:],
                                        in1=e1b, op=MULT)
                nc.gpsimd.dma_start(out=yv[:, 4 * g:4 * g + 4, :],
                                    in_=stage[:])

        # ---- software-pipelined emission -------------------------------
        # A0 A1 H0 | A2 C0 A3 C1 H1 | A4 C2 A5 C3 H2 | A6 C4 A7 C5 H3 | C6 C7
        emit_A1(0)
        emit_A2(0)
        emit_A1(1)
        emit_A2(1)
        emit_H(0)
        for q in range(1, NG):
            emit_A1(2 * q)
            emit_C(2 * q - 2)
            emit_A2(2 * q)
            emit_A1(2 * q + 1)
            emit_C(2 * q - 1)
            emit_A2(2 * q + 1)
            emit_H(q)
        emit_C(B_LOC - 2)
        emit_C(B_LOC - 1)

    nc.compile()
    return nc


_CACHE = {}
PROFILE = False
LAST_RESULT = None


def _host_constants(initial_level, tau):
    tau_c = np.maximum(tau.astype(np.float64), 1e-8)
    a = DT / tau_c
    e = np.exp(-a)
    em1 = 1.0 - e
    cEta = e * a * em1
    cS = em1 - e * a
    cEtp = a * em1  # cEta / e
    lvl = initial_level.astype(np.float64)

    i = np.arange(P, dtype=np.float64)[:, None]
    r = np.arange(P, dtype=np.float64)
    einv = np.exp(a[None, :] * i)               # e^{-i}
    G1 = cS[None, :] * einv
    G2 = cEtp[None, :] * einv
    E = np.exp(-a[None, :] * i)
    e1_ = E.astype(np.float32)
    gc_ = np.concatenate([G1, G2], axis=1).astype(np.float32)

    w_tril = (i <= r[None, :]).astype(np.float64)
    w_trilM = w_tril * (r[None, :] - i)
    wsum = np.zeros((P, P), dtype=np.float64)
    for m in range(NBLK):
        wsum[:, 16 * m + m] = 1.0
        wsum[:, 16 * m + 8 + m] = i[:, 0]
    wmat = np.concatenate([w_tril, w_trilM, wsum], axis=1).astype(np.float32)

    wr_ = np.zeros((16, NBLK * P), dtype=np.float64)
    for m in range(NBLK):
        wr_[m, m * P:(m + 1) * P] = 1.0
        wr_[8 + m, m * P:(m + 1) * P] = r
    wr_ = wr_.astype(np.float32)

    eL = e ** 128
    e127 = e ** 127
    e126 = e ** 126
    cs0 = lvl / em1
    ce0 = lvl / (em1 * em1)
    consts = [
        e127 / cEtp,           # 0 k1
        127.0 * e126 / cEtp,   # 1 k2
        e126 / cEtp,           # 2 k3
        128.0 * e127,          # 3 bL
        e * cEta,              # 4 mu1
        e * cS + cEta,         # 5 mu2
        cEta,                  # 6 nu
        np.zeros_like(e),      # 7 (unused)
        np.zeros_like(e),      # 8 (unused)
        cs0,                   # 9
        ce0,                   # 10
    ]
    chc = np.zeros((P, 11 * NBC + NBC * NBLK), dtype=np.float64)
    for bc in range(NBC):
        c = bc % KC
        sl = slice(c * P, (c + 1) * P)
        for j, v in enumerate(consts):
            chc[:, j * NBC + bc] = v[sl]
        for m in range(NBLK):
            chc[:, 11 * NBC + bc * NBLK + m] = 0.0 if m == 0 else eL[sl]
    chc = chc.astype(np.float32)

    ident = np.eye(P, dtype=np.float32)
    return e1_, gc_, wmat, wr_, ident, chc


def kernel(inputs, initial_level, tau):
    global LAST_RESULT
    inputs = np.ascontiguousarray(np.asarray(inputs, dtype=np.float32))
    initial_level = np.asarray(initial_level, dtype=np.float32)
    tau = np.asarray(tau, dtype=np.float32)
    assert inputs.shape == (B, T, K), inputs.shape

    e1_, gc_, wmat, wr_, ident, chc = _host_constants(initial_level, tau)

    if "nc" not in _CACHE:
        _CACHE["nc"] = build_nc()
    nc = _CACHE["nc"]

    in_maps = [
        {
            "x": inputs[i * B_LOC: (i + 1) * B_LOC],
            "e1": e1_,
            "gc": gc_,
            "wmat": wmat,
            "wr": wr_,
            "ident": ident,
            "chc": chc,
        }
        for i in range(N_CORES)
    ]
    res = run_bass_kernel_spmd(nc, in_maps, list(range(N_CORES)), trace=PROFILE)
    LAST_RESULT = res
    return np.concatenate([r["y"] for r in res.results], axis=0)


# revision 16
# speedup vs baseline: 1.3451x; 1.3451x over previous
"""Alpha-filter (keras_spiking AlphaCell) Trainium2 Bass kernel.

Math: per (batch b, feature k) the reference runs the 2-state recurrence
    x_t = A_k x_{t-1} + B_k u_t,   y_t = x_t[1]
with A_k = e*[[1-a, -a/tau],[dt, 1+a]], a = dt/tau, e = exp(-a).
A_k has a defective double eigenvalue e (A = e(I+N), N nilpotent), so the
recurrence reduces to two CHAINED first-order scans (scan2 consumes scan1's
output directly — no intermediate tensor):

    s_t   = e * s_{t-1} + u_t          s_0   = L/(1-e)      (L = initial_level)
    eta_t = e * eta_{t-1} + s_{t-1}    eta_0 = L/(1-e)^2
    y_t   = [e*a*(1-e)] * eta_t + [(1-e) - e*a] * s_t

Each scan is one DVE tensor_tensor_scan over a [128 features, T] tile
(time on the free dim); the per-feature multiplier stream is a stride-0
broadcast AP of a [128,1] column (verified exact on HW).  Input tiles
arrive in [time, features] layout (contiguous DMA) and are transposed on
the PE into PSUM; scan1 reads PSUM directly.  The combine is two in-place
ScalarE pre-scales followed by PE transpose-back pairs accumulating in
PSUM; ScalarE copies y to SBUF for the contiguous store.

Sharding: data-parallel over batch, 8 batches per core x 8 cores.
"""

import sys

for _p in ("/opt/trn_rl_repo",):
    if _p not in sys.path:
        sys.path.insert(0, _p)

from contextlib import ExitStack

import numpy as np

import concourse.bacc as bacc
import concourse.bass as bass
import concourse.tile as tile
from concourse import mybir
from concourse.bass_utils import run_bass_kernel_spmd

DT = 0.001
B, T, K = 64, 1024, 512
N_CORES = 8
B_LOC = B // N_CORES  # 8 batches per core
P = 128
KC = K // P   # 4 feature chunks of 128
TCH = T // P  # 8 time chunks of 128

F32 = mybir.dt.float32
MULT = mybir.AluOpType.mult
ADD = mybir.AluOpType.add


def _bcast(col_ap, n):
    """[P,1] AP -> [P,n] stride-0 free-dim broadcast AP."""
    return bass.AP(tensor=col_ap.tensor, offset=col_ap.offset, ap=[col_ap.ap[0], [0, n]])


def build_nc():
    nc = bacc.Bacc(None, target_bir_lowering=False)

    x = nc.dram_tensor("x", [B_LOC, T, K], F32, kind="ExternalInput")
    # cols[c] = [e, s0, eta0, c_eta, c_s] per feature chunk
    cols = nc.dram_tensor("cols", [KC, 5, P], F32, kind="ExternalInput")
    ident = nc.dram_tensor("ident", [P, P], F32, kind="ExternalInput")
    y = nc.dram_tensor("y", [B_LOC, T, K], F32, kind="ExternalOutput")

    with tile.TileContext(nc) as tc, ExitStack() as ctx:
        singles = ctx.enter_context(tc.tile_pool(name="singles", bufs=1))
        inpool = ctx.enter_context(tc.tile_pool(name="inpool", bufs=3))
        outpool = ctx.enter_context(tc.tile_pool(name="outpool", bufs=2))
        epool = ctx.enter_context(tc.tile_pool(name="epool", bufs=2 * KC))
        psum_u = ctx.enter_context(tc.tile_pool(name="psum_u", bufs=2, space="PSUM"))
        psum_y = ctx.enter_context(tc.tile_pool(name="psum_y", bufs=4, space="PSUM"))

        # ---- one-time constant loads -----------------------------------
        def load_col(idx, name):
            t = singles.tile([P, KC], F32, tag=name)
            nc.scalar.dma_start(out=t[:], in_=cols.rearrange("c s p -> p c s")[:, :, idx])
            return t

        ident_t = singles.tile([P, P], F32)
        nc.sync.dma_start(out=ident_t[:], in_=ident[:])
        e_col = load_col(0, "e_col")
        s0_col = load_col(1, "s0_col")
        eta0_col = load_col(2, "eta0_col")
        ceta_col = load_col(3, "ceta_col")
        cs_col = load_col(4, "cs_col")

        # PE warm-up during the initial DMA window: HAM needs ~3.4us of
        # activity before the PE clock doubles.  Plain matmuls on memset
        # scratch have no DMA dependency, so they start at t~0 and finish
        # before the first real transpose is ready.
        scratch = singles.tile([P, P], F32)
        nc.gpsimd.memset(scratch[:], 0.0)
        warm = psum_y.tile([P, K], F32, name="warm", tag="yp")
        for i in range(6):
            nc.tensor.matmul(
                warm[:, 0:P], scratch[:], scratch[:], start=True, stop=True
            )

        # static double-buffered s tiles: col 0 = s0 written once per tile
        s_static = [
            [
                singles.tile(
                    [P, T + 1], F32, tag=f"s_{c}_{par}", name=f"s_{c}_{par}"
                )
                for par in range(2)
            ]
            for c in range(KC)
        ]
        for c in range(KC):
            for par in range(2):
                nc.scalar.copy(s_static[c][par][:, 0:1], s0_col[:, c : c + 1])

        # ---- main loop over local batches ------------------------------
        for b in range(B_LOC):
            par = b % 2
            # staged input: in_stage[p, tch, k] = x[b, tch*128+p, k]
            # split DMA per t-chunk group so transposes can start early
            in_stage = inpool.tile([P, TCH, K], F32)
            xv = x[b].rearrange("(a p) k -> p a k", p=P)
            for h in range(0, TCH, 4):
                nc.sync.dma_start(
                    out=in_stage[:, h : h + 4, :],
                    in_=xv[:, h : h + 4, :],
                )

            # phase A: transposes + scan1 for every chunk, then phase B:
            # scan2 + pre-scales.  Issuing all scan1s first keeps each
            # scan2's RAW dependency several DVE instructions ahead, so the
            # engine never bubbles on a just-finished producer.
            s_fulls = []
            for c in range(KC):
                # transpose u into [128 features, T] (PSUM), time along free
                uT = psum_u.tile([P, T], F32)
                for t in range(TCH):
                    nc.tensor.transpose(
                        uT[:, t * P : (t + 1) * P],
                        in_stage[:, t, c * P : (c + 1) * P],
                        ident_t[:],
                    )
                s_full = s_static[c][par]
                nc.vector.tensor_tensor_scan(
                    out=s_full[:, 1 : T + 1],
                    data0=_bcast(e_col[:, c : c + 1], T),
                    data1=uT[:],
                    initial=s0_col[:, c : c + 1],
                    op0=MULT,
                    op1=ADD,
                )
                s_fulls.append(s_full)

            s_tiles = []
            eta_tiles = []
            for c in range(KC):
                s_full = s_fulls[c]
                eta = epool.tile([P, T], F32)
                nc.vector.tensor_tensor_scan(
                    out=eta[:],
                    data0=_bcast(e_col[:, c : c + 1], T),
                    data1=s_full[:, 0:T],
                    initial=eta0_col[:, c : c + 1],
                    op0=MULT,
                    op1=ADD,
                )
                # pre-scales (eta in place; scan2 already consumed s[:,0:T],
                # so the shat read of s[:,1:] is dep-safe).  ScalarE normally;
                # the last batch uses the scan-idle VectorE to shorten the
                # tail's serial chain.
                shat = epool.tile([P, T], F32, tag="shat")
                if b == B_LOC - 1:
                    nc.vector.tensor_scalar_mul(eta[:], eta[:], ceta_col[:, c : c + 1])
                    nc.vector.tensor_scalar_mul(
                        shat[:], s_full[:, 1 : T + 1], cs_col[:, c : c + 1]
                    )
                else:
                    nc.scalar.mul(eta[:], eta[:], ceta_col[:, c : c + 1])
                    nc.scalar.mul(
                        shat[:], s_full[:, 1 : T + 1], cs_col[:, c : c + 1]
                    )
                s_tiles.append(shat)
                eta_tiles.append(eta)

            # transpose back; the combine is the PSUM accumulation of the
            # two pre-scaled transposes: y[t,k] = eta_hat[k,t] + s_hat[k,t]
            out_stage = outpool.tile([P, TCH, K], F32)
            yv = y[b].rearrange("(a p) k -> p a k", p=P)
            for t in range(TCH):
                yp = psum_y.tile([P, K], F32)
                for c in range(KC):
                    nc.tensor.matmul(
                        yp[:, c * P : (c + 1) * P],
                        eta_tiles[c][:, t * P : (t + 1) * P],
                        ident_t[:],
                        is_transpose=True,
                        start=True,
                        stop=False,
                    )
                    nc.tensor.matmul(
                        yp[:, c * P : (c + 1) * P],
                        s_tiles[c][:, t * P : (t + 1) * P],
                        ident_t[:],
                        is_transpose=True,
                        start=False,
                        stop=True,
                    )
                if b == B_LOC - 1 and t % 2 == 0:
                    nc.vector.tensor_copy(out_stage[:, t, :], yp[:])
                else:
                    nc.scalar.copy(out_stage[:, t, :], yp[:])
                if b == B_LOC - 1:
                    # spread the tail stores over three HWDGE queues so the
                    # final DMAs drain in parallel
                    eng = (nc.sync, nc.scalar)[t % 2]
                    eng.dma_start(out=yv[:, t, :], in_=out_stage[:, t, :])
                elif t % 2 == 1:
                    h = t // 2
                    nc.sync.dma_start(
                        out=yv[:, h * 2 : (h + 1) * 2, :],
                        in_=out_stage[:, h * 2 : (h + 1) * 2, :],
                    )

    nc.compile()
    return nc


_CACHE = {}
PROFILE = False
LAST_RESULT = None


def _host_constants(initial_level, tau):
    tau_c = np.maximum(tau.astype(np.float64), 1e-8)
    a = DT / tau_c
    e = np.exp(-a)
    em1 = 1.0 - e
    ea = e * a
    s0 = initial_level.astype(np.float64) / em1
    eta0 = initial_level.astype(np.float64) / (em1 * em1)
    c_eta = ea * em1
    c_s = em1 - ea
    cols = np.stack(
        [
            e.astype(np.float32).reshape(KC, P),
            s0.astype(np.float32).reshape(KC, P),
            eta0.astype(np.float32).reshape(KC, P),
            c_eta.astype(np.float32).reshape(KC, P),
            c_s.astype(np.float32).reshape(KC, P),
        ],
        axis=1,
    )  # [KC, 5, P]
    ident = np.eye(P, dtype=np.float32)
    return cols, ident


def kernel(inputs, initial_level, tau):
    global LAST_RESULT
    inputs = np.ascontiguousarray(np.asarray(inputs, dtype=np.float32))
    initial_level = np.asarray(initial_level, dtype=np.float32)
    tau = np.asarray(tau, dtype=np.float32)
    assert inputs.shape == (B, T, K), inputs.shape

    cols, ident = _host_constants(initial_level, tau)

    if "nc" not in _CACHE:
        _CACHE["nc"] = build_nc()
    nc = _CACHE["nc"]

    in_maps = [
        {
            "x": inputs[i * B_LOC : (i + 1) * B_LOC],
            "cols": cols,
            "ident": ident,
        }
        for i in range(N_CORES)
    ]
    res = run_bass_kernel_spmd(nc, in_maps, list(range(N_CORES)), trace=PROFILE)
    LAST_RESULT = res
    return np.concatenate([r["y"] for r in res.results], axis=0)

